# revision 2
# baseline (speedup 1.0000x reference)
"""Inverse Haar DWT2 (pywt 'haar' idwt2 convention) on 8 Trainium2 cores.

Input  x: [16, 256, 128, 128] f32 — 4 stacked subbands (LL|LH|HL|HH) of 64
channels each.  Output: [16, 64, 256, 256] f32.

Sharding: batch dim (16) split across 8 cores, 2 batches per core.  The
transform is elementwise per (batch, channel) — no communication.

Per-core kernel (x_loc [2, 256, 128, 128] -> y_loc [2, 64, 256, 256]):
for each (batch, channel-group of N_CH):
  - 4 DMAs (one per subband) load an SBUF tile T laid out [h=128 part][band][c][w]
  - stage 1 (DVE tensor_tensor): U0|V0 = (LL|HL) + (LH|HH), U1|V1 = (LL|HL) - (LH|HH)
  - stage 2 (custom DVE LN_BWD_DX_ANT, (in0 - in1*s0 - s1)*imm2):
      out[2i+r, 2j+s] = (U_r +- V_r) * 0.5 written with stride-2 column
      interleave into an SBUF tile laid out [i=128 part][r][c][col]
  - 2 DMAs (one per output row phase r) store rows 2i+r with row-stride 2
HBM traffic per core = 33.5 MB in + 33.5 MB out -> ~190 us roofline at
~358 GB/s per-NC HBM bandwidth.

This container's walrus build supports only ONE semaphore wait per
instruction; Tile emits multi-wait instructions (incl. the final drain), so
after TileContext exit we redistribute extra waits onto single-wait NOPs
inserted before the instruction on the same engine.
"""

import numpy as np

import concourse.bass as bass
import concourse.mybir as mybir
from concourse.tile import TileContext
from concourse.bass_utils import run_bass_kernel_spmd

N_CORES = 8
B, C4, H, W = 16, 256, 128, 128
CH = C4 // 4          # 64 output channels
B_LOC = B // N_CORES  # 2 batches per core
N_CH = 8              # channels per tile iteration
F32 = mybir.dt.float32


def _split_multi_waits(nc):
    """Move extra semaphore waits onto single-wait NOPs placed immediately
    before the over-subscribed instruction (same engine, so per-engine
    program order is preserved)."""
    n_split = 0
    for f in nc.m.functions:
        for blk in f.blocks:
            il = blk.instructions
            new_list = []
            for inst in il:
                si = getattr(inst, "sync_info", None)
                ow = si.on_wait if si is not None else None
                if ow and len(ow) > 1:
                    extra = list(ow[:-1])
                    del ow[:-1]
                    for w in extra:
                        n_split += 1
                        new_list.append(
                            mybir.InstNoOp(
                                name=f"{inst.name}-waitsplit-{n_split}",
                                engine=inst.engine,
                                sync_info=mybir.SyncInfo(on_wait=[w], on_update=[]),
                            )
                        )
                new_list.append(inst)
            il[:] = new_list
    return n_split


def _build_kernel():
    nc = bass.Bass("TRN2")
    x = nc.dram_tensor("x", [B_LOC, C4, H, W], F32, kind="ExternalInput")
    y = nc.dram_tensor("y", [B_LOC, CH, 2 * H, 2 * W], F32, kind="ExternalOutput")

    with TileContext(nc) as tc:
        with (
            tc.tile_pool(name="tin", bufs=3) as pin,
            tc.tile_pool(name="tuv", bufs=2) as puv,
            tc.tile_pool(name="tout", bufs=3) as pout,
        ):
            for b in range(B_LOC):
                for cg in range(CH // N_CH):
                    cs = cg * N_CH
                    # ---- load: [p=h][band][c][w]
                    T = pin.tile([128, 4 * N_CH * W], F32, tag="T")
                    Tv = T[:].rearrange("p (band c w) -> p band c w", band=4, c=N_CH)
                    for band in range(4):
                        nc.sync.dma_start(
                            out=Tv[:, band],
                            in_=x[b, band * CH + cs : band * CH + cs + N_CH].rearrange(
                                "c h w -> h c w"
                            ),
                        )
                    # ---- stage 1: vertical butterfly
                    # band index = b1*2 + b0: LL=00 LH=01 HL=10 HH=11
                    # in0 = (LL, HL) [b0=0], in1 = (LH, HH) [b0=1]
                    UV = puv.tile([128, 4 * N_CH * W], F32, tag="UV")  # [r][U|V][c][w]
                    Tb = T[:].rearrange("p (b1 b0 x) -> p b1 b0 x", b1=2, b0=2)
                    in0 = Tb[:, :, 0]
                    in1 = Tb[:, :, 1]
                    UVr = UV[:].rearrange("p (r x) -> p r x", r=2)
                    out0 = UVr[:, 0].rearrange("p (pair x) -> p pair x", pair=2)
                    out1 = UVr[:, 1].rearrange("p (pair x) -> p pair x", pair=2)
                    nc.vector.tensor_add(out=out0, in0=in0, in1=in1)  # U0|V0
                    nc.vector.tensor_sub(out=out1, in0=in0, in1=in1)  # U1|V1
                    # ---- stage 2: horizontal butterfly, column interleave
                    OUT = pout.tile([128, 2 * N_CH * 2 * W], F32, tag="OUT")
                    OUTv = OUT[:].rearrange(
                        "p (r c j s) -> p r c j s", r=2, c=N_CH, j=W, s=2
                    )
                    UVv = UV[:].rearrange("p (r pair c w) -> p r pair c w", r=2, pair=2, c=N_CH)
                    for r in range(2):
                        u = UVv[:, r, 0]
                        v = UVv[:, r, 1]
                        nc.vector.tensor_add(out=OUTv[:, r, :, :, 0], in0=u, in1=v)
                        nc.vector.tensor_sub(out=OUTv[:, r, :, :, 1], in0=u, in1=v)
                    # ---- * 0.5 in place on the contiguous tile (ScalarE/ACT)
                    nc.scalar.mul(OUT[:], OUT[:], 0.5)
                    # ---- store: rows 2i+r, contiguous 1KB runs, row-stride 2
                    OUTr = OUT[:].rearrange("p (r x) -> p r x", r=2)
                    yv = y[b, cs : cs + N_CH].rearrange("c (i r) w -> r i c w", r=2)
                    for r in range(2):
                        nc.sync.dma_start(
                            out=yv[r],
                            in_=OUTr[:, r].rearrange("p (c w) -> p c w", c=N_CH),
                        )

    _split_multi_waits(nc)
    return nc


_NC_CACHE = None


def _get_nc():
    global _NC_CACHE
    if _NC_CACHE is None:
        _NC_CACHE = _build_kernel()
    return _NC_CACHE


def run_sharded(x, trace=False, **kwargs):
    assert x.shape == (B, C4, H, W) and x.dtype == np.float32
    nc = _get_nc()
    in_maps = [
        {"x": np.ascontiguousarray(x[i * B_LOC : (i + 1) * B_LOC])}
        for i in range(N_CORES)
    ]
    res = run_bass_kernel_spmd(
        nc, in_maps, core_ids=list(range(N_CORES)), trace=trace, **kwargs
    )
    out = np.concatenate([r["y"] for r in res.results], axis=0)
    return out, res


def kernel(x):
    out, _ = run_sharded(np.asarray(x))
    return out
